# revision 1
# baseline (speedup 1.0000x reference)
"""Multi-head self-attention (b=4, n=2048, f=1024, h=16) on 8 trn2 NeuronCores.

Sharding: core c -> batch c//2, head-half c%2 (8 heads of 64 dims each).
Each core computes its 8 heads' attention and a partial output projection
(attn_slice @ Wo_rows); host sums the two partials per batch and adds bo.

Device dataflow per core (all matmul operands bf16, PSUM fp32):
  qT/kT  = (x@Wq+bq)^T, (x@Wk+bk)^T  laid out [feat, tok]    (W stationary)
  v      = x@Wv+bv                   laid out [tok, feat]    (xT stationary)
  S1     = [v | 1]            per-head stationaries [tok, 65]
  S0     = e^{-m} * [v | 1]
  logitsT[j, i] = k_j . q_i   (keys on partitions, 2 heads row-packed in PE)
  Etil   = exp(logitsT/32 + m_j)     (ACT bias folds the additive mask for
                                      m_i=1 queries multiplicatively)
  A1/D1  = S1^T @ Etil  (masked numerator + denominator, ones-column trick)
  A0/D0  = S0^T @ Etil  (unmasked variant; e^{-m_j} undoes the bias)
  out_i  = m_i ? A1/D1 : A0/D0   (per-column select via host mask rows)
"""

import sys

sys.path.insert(0, "/opt/trn_rl_repo")

import numpy as np
import ml_dtypes

import concourse.bass as bass
import concourse.bacc as bacc
import concourse.mybir as mybir
import concourse.tile as tile
from concourse import bass_utils

BF16 = mybir.dt.bfloat16
F32 = mybir.dt.float32
NPBF16 = ml_dtypes.bfloat16

B, N, F, H, HD = 4, 2048, 1024, 16, 64
FH = 512          # features per core (8 heads)
NC_ = 8           # cores
NTOKC = N // 128  # 16 token chunks
NIBLK = N // 512  # 4 query blocks
NJ = N // 128     # 16 key chunks
NPAIR = 4         # head pairs per core
EXPFN = mybir.ActivationFunctionType.Exp


def _emit(nc, tc, d, sorted_mode):
    """Emit the whole per-core program under TileContext tc.

    d: dict of dram tensor APs by name.
    sorted_mode: tokens are host-sorted by mask desc, with the 0/1 boundary
    inside query blocks 1..2 — blocks 0 and 3 run a single AV variant.
    """
    consts = tc.alloc_tile_pool(name="consts", bufs=1)
    persist = tc.alloc_tile_pool(name="persist", bufs=1)

    # ---- persistent activations ----------------------------------------
    qT_sb = persist.tile([128, 4 * N], BF16)   # [feat, tok], chunk fc at cols fc*N
    kT_sb = persist.tile([128, 4 * N], BF16)
    s1_sb = persist.tile([128, NJ * 8 * 65], BF16)  # per (jc, head): [v | 1]
    s0_sb = persist.tile([128, NJ * 8 * 65], BF16)  # e^{-m} * [v | 1]
    attnT = persist.tile([128, 4 * N], BF16)   # normalized attn, [feat, tok]

    # ================= phase 1: projections ==============================
    with tc.tile_pool(name="p1sb", bufs=1) as p1sb, \
         tc.tile_pool(name="pkt", bufs=1, space="PSUM") as pkt:
        # xT + Wk loads first (kT matmuls consume them chunk by chunk)
        xT_sb = p1sb.tile([128, 8 * N], BF16)
        wk_sb = p1sb.tile([128, 8 * FH], BF16)
        bqk = consts.tile([128, 8], F32)       # bq chunks (0-3), bk chunks (4-7)
        nc.sync.dma_start(out=bqk, in_=d["bqk"])
        for fc in range(8):
            nc.sync.dma_start(
                out=wk_sb[:, fc * FH:(fc + 1) * FH],
                in_=d["wk"][fc * 128:(fc + 1) * 128, :],
            )
            nc.sync.dma_start(
                out=xT_sb[:, fc * N:(fc + 1) * N],
                in_=d["xT"][fc * 128:(fc + 1) * 128, :],
            )

        # kT: fc-outer over 4 concurrent psum tiles (full PSUM) so the first
        # matmuls start as soon as chunk 0 of xT/Wk lands.
        for grp in range(2):
            pks = [
                pkt.tile([128, 1024], F32, tag=f"pp{t}", name=f"pk{t}")
                for t in range(4)
            ]
            for fc in range(8):
                for t in range(4):
                    fhc, half = grp * 2 + t // 2, t % 2
                    lhsT = wk_sb[:, fc * FH + fhc * 128: fc * FH + (fhc + 1) * 128]
                    for nn in range(2):
                        off = half * 1024 + nn * 512
                        nc.tensor.matmul(
                            pks[t][:, nn * 512:(nn + 1) * 512],
                            lhsT,
                            xT_sb[:, fc * N + off: fc * N + off + 512],
                            start=(fc == 0),
                            stop=(fc == 7),
                        )
            for t in range(4):
                fhc, half = grp * 2 + t // 2, t % 2
                nc.vector.tensor_scalar_add(
                    out=kT_sb[:, fhc * N + half * 1024: fhc * N + half * 1024 + 1024],
                    in0=pks[t][:],
                    scalar1=bqk[:, 4 + fhc: 5 + fhc],
                )

        # remaining loads (emitted after kT matmuls so they queue behind)
        mjb = consts.tile([128, NJ], F32)      # exp bias columns (m per key chunk)
        nc.sync.dma_start(out=mjb, in_=d["mjb"])
        emn = consts.tile([128, NTOKC], F32)   # e^{-m} per token chunk
        nc.sync.dma_start(out=emn, in_=d["emn"])
        mr4 = consts.tile([4, N], F32)         # rows [m, 1-m, m, 1-m]
        nc.sync.dma_start(out=mr4, in_=d["mr4"])
        mrp1 = consts.tile([2, N], F32)        # rows [m, m]
        nc.sync.dma_start(out=mrp1, in_=d["mrp"][0:2, :])
        mrp0 = consts.tile([2, N], F32)        # rows [1-m, 1-m]
        nc.sync.dma_start(out=mrp0, in_=d["mrp"][2:4, :])
        bvb = consts.tile([128, FH], F32)      # bv broadcast over partitions
        nc.sync.dma_start(out=bvb, in_=d["bvb"])
        wo_sb = consts.tile([128, 4 * 1024], BF16)
        for fc in range(4):
            nc.sync.dma_start(
                out=wo_sb[:, fc * 1024:(fc + 1) * 1024],
                in_=d["wo"][fc * 128:(fc + 1) * 128, :],
            )
        w_sb = {}
        for wname in ("wq", "wv"):
            t = p1sb.tile([128, 8 * FH], BF16, tag=wname)
            for fc in range(8):
                nc.sync.dma_start(
                    out=t[:, fc * FH:(fc + 1) * FH],
                    in_=d[wname][fc * 128:(fc + 1) * 128, :],
                )
            w_sb[wname] = t

        # qT (xT is resident by now; fc-inner keeps PSUM small)
        for fhc in range(4):
            for half in range(2):
                pk = pkt.tile(
                    [128, 1024], F32, tag=f"pp{(fhc * 2 + half) % 2}", name="pkq"
                )
                for fc in range(8):
                    lhsT = w_sb["wq"][:, fc * FH + fhc * 128: fc * FH + (fhc + 1) * 128]
                    for nn in range(2):
                        off = half * 1024 + nn * 512
                        nc.tensor.matmul(
                            pk[:, nn * 512:(nn + 1) * 512],
                            lhsT,
                            xT_sb[:, fc * N + off: fc * N + off + 512],
                            start=(fc == 0),
                            stop=(fc == 7),
                        )
                nc.vector.tensor_scalar_add(
                    out=qT_sb[:, fhc * N + half * 1024: fhc * N + half * 1024 + 1024],
                    in0=pk[:],
                    scalar1=bqk[:, fhc: fhc + 1],
                )

        # v: out[tok_chunk 128, fh 512] = xT_chunk^T @ Wv ; then build S1/S0
        for tokc in range(NTOKC):
            pv = pkt.tile([128, FH], F32, tag=f"pp{2 + tokc % 2}", name="pv")
            for fc in range(8):
                nc.tensor.matmul(
                    pv[:],
                    xT_sb[:, fc * N + tokc * 128: fc * N + (tokc + 1) * 128],
                    w_sb["wv"][:, fc * FH:(fc + 1) * FH],
                    start=(fc == 0),
                    stop=(fc == 7),
                )
            base = tokc * 8 * 65
            s1_v = s1_sb[:, base:base + 8 * 65].rearrange("p (h c) -> p h c", h=8)
            s0_v = s0_sb[:, base:base + 8 * 65].rearrange("p (h c) -> p h c", h=8)
            pv_v = pv[:].rearrange("p (h c) -> p h c", h=8)
            bv_v = bvb[:].rearrange("p (h c) -> p h c", h=8)
            # S1 = v + bv (head-strided dest, ones col at c=64)
            nc.vector.tensor_add(out=s1_v[:, :, 0:64], in0=pv_v, in1=bv_v)
            nc.vector.memset(s1_v[:, :, 64:65], 1.0)
            # S0 = e^{-m} * S1
            nc.vector.tensor_scalar_mul(
                out=s0_v[:, :, 0:64],
                in0=s1_v[:, :, 0:64],
                scalar1=emn[:, tokc:tokc + 1],
            )
            emn_b = bass.AP(
                tensor=emn.tensor,
                offset=emn[:, tokc:tokc + 1].offset,
                ap=[emn[:, tokc:tokc + 1].ap[0], [0, 8], [1, 1]],
            )
            nc.vector.tensor_copy(out=s0_v[:, :, 64:65], in_=emn_b)

    # ================= phase 2: attention ================================
    with tc.tile_pool(name="pP", bufs=2, space="PSUM") as pP, \
         tc.tile_pool(name="pacc", bufs=1, space="PSUM") as pacc, \
         tc.tile_pool(name="sexp", bufs=4) as sexp, \
         tc.tile_pool(name="episb", bufs=2) as episb, \
         tc.tile_pool(name="osb", bufs=3) as osb, \
         tc.tile_pool(name="epidr", bufs=2, space="DRAM") as epidr:

        # O-projection micro-ops (one instruction each). In sorted mode they
        # are drained into the pure query blocks' PE slack, using the acc
        # slots the single-variant blocks leave free.
        pending = []

        def o_ops_for_iblk(ib, tags, fin_act=False):
            ops = []
            from itertools import cycle
            tagc = cycle(tags)
            for tokc in range(ib * 4, ib * 4 + 4):
                for half in range(2):
                    st = {}
                    for fc in range(4):
                        def mm(fc=fc, tokc=tokc, half=half, st=st):
                            if fc == 0:
                                st["po"] = pacc.tile(
                                    [128, 512], F32, tag=next(tagc), name="po"
                                )
                            nc.tensor.matmul(
                                st["po"][:],
                                attnT[:, fc * N + tokc * 128: fc * N + (tokc + 1) * 128],
                                wo_sb[:, fc * 1024 + half * 512: fc * 1024 + half * 512 + 512],
                                start=(fc == 0),
                                stop=(fc == 3),
                            )
                        ops.append(mm)

                    def fin(tokc=tokc, half=half, st=st):
                        ot = osb.tile([128, 512], F32, tag="ot", name="ot")
                        if fin_act:
                            # tail runs after all exps: ScalarE is idle and
                            # the DVE queue is busy with the last epilogue
                            nc.scalar.activation(
                                out=ot, in_=st["po"][:],
                                func=mybir.ActivationFunctionType.Copy,
                            )
                        else:
                            nc.vector.tensor_copy(out=ot, in_=st["po"][:])
                        nc.sync.dma_start(
                            out=d["y"][tokc * 128:(tokc + 1) * 128,
                                       half * 512:(half + 1) * 512],
                            in_=ot,
                        )
                    ops.append(fin)
            return ops

        iblk_order = [1, 2, 0, NIBLK - 1] if sorted_mode else list(range(NIBLK))
        for iblk in iblk_order:
            # variant v: 0/2 = masked (A1) for heads A/B, 1/3 = unmasked (A0).
            # With host-sorted tokens, query block 0 is all m_i=1 and block 3
            # all m_i=0, so those need only one AV variant.
            if sorted_mode and iblk == 0:
                active = [0, 2]
            elif sorted_mode and iblk == NIBLK - 1:
                active = [1, 3]
            else:
                active = [0, 1, 2, 3]
            for pair in range(NPAIR):
                accs = {
                    v: pacc.tile([65, 512], F32, tag=f"acc{v}", name=f"acc{v}")
                    for v in active
                }
                def qk(j):
                    P = pP.tile([128, 1024], F32, tag="logits")
                    for hl, tp in ((0, 0), (1, 64)):
                        nc.tensor.matmul(
                            P[:, hl * 512:(hl + 1) * 512],
                            kT_sb[tp:tp + 64, pair * N + j * 128: pair * N + (j + 1) * 128],
                            qT_sb[tp:tp + 64, pair * N + iblk * 512: pair * N + (iblk + 1) * 512],
                            start=True,
                            stop=True,
                            tile_position=(tp, 0),
                        )
                    return P

                # software-pipelined emission, QK two iterations ahead: the PE
                # order per period is [QK(j+2); AV(j)], so QK(j+1) always
                # finishes long before exp(j+1) needs it and the exp chain
                # never waits on a matmul.
                P0 = qk(0)
                P1 = qk(1)
                Ptil = {0: P0, 1: P1}
                for j in range(NJ):
                    S = sexp.tile([128, 1024], BF16, tag="etil")
                    nc.scalar.activation(
                        out=S[:], in_=Ptil.pop(j), func=EXPFN,
                        bias=mjb[:, j:j + 1], scale=1.0 / 32.0,
                    )
                    if j + 2 < NJ:
                        Ptil[j + 2] = qk(j + 2)
                    for hl in range(2):
                        hcore = 2 * pair + hl
                        soff = j * 8 * 65 + hcore * 65
                        rhs = S[:, hl * 512:(hl + 1) * 512]
                        for v, s_sb in ((2 * hl, s1_sb), (2 * hl + 1, s0_sb)):
                            if v in accs:
                                nc.tensor.matmul(
                                    accs[v][:], s_sb[:, soff:soff + 65], rhs,
                                    start=(j == 0), stop=(j == NJ - 1),
                                )
                    # pure blocks have PE slack and 2 free acc slots: drain
                    # O-projection micro-ops for already-finished blocks.
                    if len(active) == 2:
                        hold = 16 if iblk == iblk_order[-1] else 0
                        for _ in range(2):
                            if len(pending) > hold:
                                pending.pop(0)()

                # ---- epilogue: select + normalize -----------------------
                na = len(active)
                # the very last pair's staging copies would queue behind the
                # previous pair's epilogue on the in-order DVE; ScalarE is
                # idle once the final exp has issued, so use it there
                last_pair = iblk == iblk_order[-1] and pair == NPAIR - 1
                asb = {}
                for v in active:
                    t = episb.tile([65, 512], F32, tag=f"asb{v}", name=f"asb{v}")
                    if last_pair:
                        nc.scalar.activation(
                            out=t, in_=accs[v][:],
                            func=mybir.ActivationFunctionType.Copy,
                        )
                    else:
                        nc.vector.tensor_copy(out=t, in_=accs[v][:])
                    asb[v] = t
                rin = episb.tile([4, 512], F32, tag="rin")
                for k, v in enumerate(active):
                    nc.sync.dma_start(out=rin[k:k + 1, :], in_=asb[v][64:65, :])
                rsc = episb.tile([4, 512], F32, tag="rsc")
                nc.vector.reciprocal_approx_fast(
                    out=rsc[0:na, :], in_=rin[0:na, :]
                )
                # mask rows matching `active`: dual -> [m,1-m,m,1-m]; pure
                # blocks -> [m,m] / [1-m,1-m] (from mrp).
                ib = iblk * 512
                if na == 4:
                    mrow = mr4[:, ib:ib + 512]
                else:
                    mrow = (mrp1 if active[0] == 0 else mrp0)[:, ib:ib + 512]
                nc.vector.tensor_mul(
                    out=rsc[0:na, :], in0=rsc[0:na, :], in1=mrow
                )
                stg2 = epidr.tile([4, 512], F32, tag="stg2")
                nc.sync.dma_start(out=stg2[0:na, :], in_=rsc[0:na, :])
                rball = episb.tile([64, 4 * 512], F32, tag="rball")
                nc.sync.dma_start(
                    out=rball[:, 0:na * 512],
                    in_=bass.AP(tensor=stg2.tensor, offset=stg2.offset,
                                ap=[[0, 64], [512, na], [1, 512]]),
                )
                rb = {
                    v: rball[:, k * 512:(k + 1) * 512]
                    for k, v in enumerate(active)
                }
                for hl in range(2):
                    dstc = pair * N + iblk * 512
                    v1, v0 = 2 * hl, 2 * hl + 1
                    if na == 4:
                        t1 = episb.tile([64, 512], F32, tag="ept1")
                        t2 = episb.tile([64, 512], F32, tag="ept2")
                        nc.vector.tensor_mul(out=t1, in0=asb[v1][0:64, :], in1=rb[v1])
                        nc.vector.tensor_mul(out=t2, in0=asb[v0][0:64, :], in1=rb[v0])
                        if hl == 0:
                            nc.vector.tensor_add(
                                out=attnT[0:64, dstc:dstc + 512], in0=t1, in1=t2
                            )
                        else:
                            t3 = episb.tile([64, 512], BF16, tag="ept3")
                            nc.vector.tensor_add(out=t3, in0=t1, in1=t2)
                            nc.sync.dma_start(
                                out=attnT[64:128, dstc:dstc + 512], in_=t3
                            )
                    else:
                        vv = v1 if v1 in asb else v0
                        if hl == 0:
                            nc.vector.tensor_mul(
                                out=attnT[0:64, dstc:dstc + 512],
                                in0=asb[vv][0:64, :], in1=rb[vv],
                            )
                        else:
                            t3 = episb.tile([64, 512], BF16, tag="ept3")
                            nc.vector.tensor_mul(out=t3, in0=asb[vv][0:64, :], in1=rb[vv])
                            nc.sync.dma_start(
                                out=attnT[64:128, dstc:dstc + 512], in_=t3
                            )

            # queue this block's O-projection. Blocks 1/2/0 drain inside the
            # pure blocks (0 and 3) on free acc slots; block 3 drains at the
            # tail on the slots block 0 used.
            if sorted_mode:
                tags = ("acc1", "acc3") if iblk in (1, 2) else ("acc0", "acc2")
                pending.extend(
                    o_ops_for_iblk(iblk, tags, fin_act=(iblk == NIBLK - 1))
                )
            else:
                pending.extend(o_ops_for_iblk(iblk, ("acc0", "acc2"), fin_act=True))

        # ===== tail: drain remaining O-projection ops ======================
        if sorted_mode:
            # The last block's O-proj waits ~13us on its epilogue chain; PE
            # would go idle past the HAM MID window and re-throttle to
            # 1.2GHz. Keep it warm with dependency-free filler matmuls that
            # occupy exactly that window.
            warm = pacc.tile([128, 512], F32, tag="acc1", name="warm")
            for _ in range(14):
                nc.tensor.matmul(
                    warm[:], wo_sb[:, 0:128], wo_sb[:, 0:512],
                    start=True, stop=True,
                )
        while pending:
            pending.pop(0)()

    persist.release()
    consts.release()


_CACHE = {}


def build_program(variant="sorted"):
    if variant in _CACHE:
        return _CACHE[variant]
    nc = bacc.Bacc("TRN2", target_bir_lowering=False, debug=False)
    d = {}
    d["xT"] = nc.dram_tensor("xT", (F, N), BF16, kind="ExternalInput").ap()
    d["wq"] = nc.dram_tensor("wq", (F, FH), BF16, kind="ExternalInput").ap()
    d["wk"] = nc.dram_tensor("wk", (F, FH), BF16, kind="ExternalInput").ap()
    d["wv"] = nc.dram_tensor("wv", (F, FH), BF16, kind="ExternalInput").ap()
    d["wo"] = nc.dram_tensor("wo", (FH, F), BF16, kind="ExternalInput").ap()
    d["bqk"] = nc.dram_tensor("bqk", (128, 8), F32, kind="ExternalInput").ap()
    d["bvb"] = nc.dram_tensor("bvb", (128, FH), F32, kind="ExternalInput").ap()
    d["mjb"] = nc.dram_tensor("mjb", (128, NJ), F32, kind="ExternalInput").ap()
    d["emn"] = nc.dram_tensor("emn", (128, NTOKC), F32, kind="ExternalInput").ap()
    d["mr4"] = nc.dram_tensor("mr4", (4, N), F32, kind="ExternalInput").ap()
    d["mrp"] = nc.dram_tensor("mrp", (4, N), F32, kind="ExternalInput").ap()
    d["y"] = nc.dram_tensor("y", (N, F), F32, kind="ExternalOutput").ap()
    with tile.TileContext(nc) as tc:
        _emit(nc, tc, d, sorted_mode=(variant == "sorted"))
    nc.compile()
    _CACHE[variant] = nc
    return nc


def make_in_maps(x, inputs_mask, Wq, bq, Wk, bk, Wv, bv, Wo, bo,
                 sorted_mode=True):
    """Host-side shard prep. All args np.float32/int32 full tensors.

    sorted_mode: per batch, tokens are permuted so mask=1 tokens come first
    (attention is permutation-equivariant when q/k/v share the permutation);
    returns the per-batch permutations for un-permuting the output.
    """
    in_maps = []
    m_all = inputs_mask.astype(np.float32)
    perms = []
    for b in range(B):
        if sorted_mode:
            perms.append(np.argsort(-m_all[b], kind="stable"))
        else:
            perms.append(np.arange(N))
    for c in range(NC_):
        b, hh = c // 2, c % 2
        cs = slice(hh * FH, (hh + 1) * FH)
        m = m_all[b][perms[b]]
        xb = x[b][perms[b]]
        im = {
            "xT": np.ascontiguousarray(xb.T).astype(NPBF16),
            "wq": Wq[:, cs].astype(NPBF16),
            "wk": Wk[:, cs].astype(NPBF16),
            "wv": Wv[:, cs].astype(NPBF16),
            "wo": np.ascontiguousarray(Wo[cs, :]).astype(NPBF16),
            "bqk": np.stack(
                [bq[cs].reshape(4, 128), bk[cs].reshape(4, 128)], axis=0
            ).reshape(8, 128).T.astype(np.float32).copy(),
            "bvb": np.broadcast_to(bv[cs], (128, FH)).astype(np.float32).copy(),
            "mjb": m.reshape(NJ, 128).T.astype(np.float32).copy(),
            "emn": np.exp(-m).reshape(NTOKC, 128).T.astype(np.float32).copy(),
            "mr4": np.stack([m, 1.0 - m, m, 1.0 - m]).astype(np.float32).copy(),
            "mrp": np.stack([m, m, 1.0 - m, 1.0 - m]).astype(np.float32).copy(),
        }
        in_maps.append(im)
    return in_maps, perms


def kernel(x, inputs_mask, Wq, bq, Wk, bk, Wv, bv, Wo, bo):
    x = np.asarray(x, dtype=np.float32)
    inputs_mask = np.asarray(inputs_mask)
    Wq, bq = np.asarray(Wq, np.float32), np.asarray(bq, np.float32)
    Wk, bk = np.asarray(Wk, np.float32), np.asarray(bk, np.float32)
    Wv, bv = np.asarray(Wv, np.float32), np.asarray(bv, np.float32)
    Wo, bo = np.asarray(Wo, np.float32), np.asarray(bo, np.float32)

    # sorted mode requires the mask-1 count per batch to land inside query
    # blocks 1..2 (always true for ~Bernoulli(0.5) masks); fall back to the
    # static dual-pass program otherwise.
    c1 = inputs_mask.astype(np.int64).sum(axis=1)
    sorted_mode = bool(np.all((c1 >= 512) & (c1 <= 3 * 512)))
    nc = build_program("sorted" if sorted_mode else "dual")
    in_maps, perms = make_in_maps(
        x, inputs_mask, Wq, bq, Wk, bk, Wv, bv, Wo, bo, sorted_mode=sorted_mode
    )
    res = bass_utils.run_bass_kernel_spmd(nc, in_maps, core_ids=list(range(NC_)))
    out = np.empty((B, N, F), dtype=np.float32)
    for b in range(B):
        out[b][perms[b]] = (
            res.results[2 * b]["y"] + res.results[2 * b + 1]["y"] + bo
        )
    return out



# revision 18
# speedup vs baseline: 1.0047x; 1.0047x over previous
"""Multi-head self-attention (b=4, n=2048, f=1024, h=16) on 8 trn2 NeuronCores.

Sharding: core c -> batch c//2, head-half c%2 (8 heads of 64 dims each).
Each core computes its 8 heads' attention and a partial output projection
(attn_slice @ Wo_rows); host sums the two partials per batch and adds bo.

v3 over the original kernel (all matmul operands bf16, PSUM fp32):
  - exp has NO bias: the additive mask term exp(m_j) is folded into the AV
    stationaries instead (s1 = e^{+m}[v|1], s0 = [v|1]). This frees the
    choice of AV stationary per query block.
  - the host permutation places tokens so query blocks 0/2/3 are pure
    (block 2's mask value mu arrives as data via a third stationary
    s2 = e^{mu m}[v|1]); only block 1 needs the dual-variant select.
    AV work: 10 matmuls per (pair, key-chunk) vs 12 before.
  - the exp stream on ScalarE (256 x [128,1024] chunks at ~1.15us each,
    ~295us total) is the critical path; PE work (~310us) runs just under
    it. Phase 1 is interleaved into the attention stream: only kT/qT
    chunk 0 and half of v run up front (~30us); the rest of v, kT/qT
    chunks 1-3, and the O-projection drain as micro-ops into the PE slack
    of the pure query blocks, so ScalarE starts exp'ing as early as
    possible and never waits.
"""

import sys

sys.path.insert(0, "/opt/trn_rl_repo")

import numpy as np
import ml_dtypes

import concourse.bass as bass
import concourse.bacc as bacc
import concourse.mybir as mybir
import concourse.tile as tile
from concourse import bass_utils

BF16 = mybir.dt.bfloat16
F32 = mybir.dt.float32
NPBF16 = ml_dtypes.bfloat16

B, N, F, H, HD = 4, 2048, 1024, 16, 64
FH = 512          # features per core (8 heads)
NC_ = 8           # cores
NTOKC = N // 128  # 16 token chunks
NIBLK = N // 512  # 4 query blocks
NJ = N // 128     # 16 key chunks
NPAIR = 4         # head pairs per core
EXPFN = mybir.ActivationFunctionType.Exp
SROW = 66         # stationary cols per head
SJ = 8 * SROW     # stationary cols per key chunk (528)
NLEADV = 8        # v token-chunks computed up front; the rest drain


def _emit(nc, tc, d, sorted_mode):
    consts = tc.alloc_tile_pool(name="consts", bufs=1)
    persist = tc.alloc_tile_pool(name="persist", bufs=1)

    # ---- persistent activations ----------------------------------------
    qT_sb = persist.tile([128, 4 * N], BF16)   # [feat, tok], fhc at cols fhc*N
    kT_sb = persist.tile([128, 4 * N], BF16)
    s0_sb = persist.tile([128, NJ * SJ], BF16)  # [j][h][66]: [vb|1]
    s1_sb = persist.tile([128, NJ * SJ], BF16)  # e^{+m} * [vb|1]
    attnT = persist.tile([128, 4 * N], BF16)   # normalized attn, [feat, tok]

    # ================= phase 1 (lead-in part) ============================
    p1sb = tc.alloc_tile_pool(name="p1sb", bufs=1)
    pkt = tc.alloc_tile_pool(name="pkt", bufs=1, space="PSUM")

    bqk = consts.tile([128, 8], F32)       # bq chunks (0-3), bk (4-7)
    nc.sync.dma_start(out=bqk, in_=d["bqk"])
    # exp table warm-up (~2.7us) while DMAs run
    warm = consts.tile([128, 8], BF16)
    nc.scalar.activation(out=warm, in_=bqk, func=EXPFN, scale=0.0)

    xT_sb = p1sb.tile([128, 8 * N], BF16)
    wk_sb = p1sb.tile([128, 8 * FH], BF16)
    wq_sb = p1sb.tile([128, 8 * FH], BF16)
    for fc in range(8):
        nc.sync.dma_start(out=wk_sb[:, fc * FH:(fc + 1) * FH],
                          in_=d["wk"][fc * 128:(fc + 1) * 128, :])
        nc.sync.dma_start(out=xT_sb[:, fc * N:(fc + 1) * N],
                          in_=d["xT"][fc * 128:(fc + 1) * 128, :])
    for fc in range(8):
        nc.sync.dma_start(out=wq_sb[:, fc * FH:(fc + 1) * FH],
                          in_=d["wq"][fc * 128:(fc + 1) * 128, :])

    pools = {"proj": pkt}  # phase 2 rebinds this to its own PSUM pool

    def proj_qk_ops(w_sb, fhc, win, bias_col, out_sb, tagsel):
        """Micro-ops for one [128,512] window of a q/k projection.

        Returns list of ("pe"|"dve", fn) micro-ops: 8 accumulation matmuls
        plus the bias-add into the bf16 destination.
        """
        st = {}
        ops = []
        grp = f"f{fhc}"
        for fc in range(8):
            def mm(fc=fc, st=st):
                if fc == 0:
                    st["pk"] = pools["proj"].tile([128, 512], F32,
                                                  tag=tagsel, name="pk")
                nc.tensor.matmul(
                    st["pk"][:],
                    w_sb[:, fc * FH + fhc * 128: fc * FH + (fhc + 1) * 128],
                    xT_sb[:, fc * N + win * 512: fc * N + (win + 1) * 512],
                    start=(fc == 0), stop=(fc == 7))
            ops.append(("pe", grp, mm))

        def fin(st=st):
            nc.vector.tensor_scalar_add(
                out=out_sb[:, fhc * N + win * 512: fhc * N + win * 512 + 512],
                in0=st["pk"][:], scalar1=bqk[:, bias_col:bias_col + 1])
        ops.append(("dve", grp, fin))
        return ops

    # kT/qT chunk 0 up front (pair 0 of the first query block)
    lead_fhc = (0,) if sorted_mode else (0, 1, 2, 3)
    for fhc in lead_fhc:
        for win in range(4):
            for _, _, op in proj_qk_ops(wk_sb, fhc, win, 4 + fhc, kT_sb,
                                        f"pp{win % 2}"):
                op()
            for _, _, op in proj_qk_ops(wq_sb, fhc, win, fhc, qT_sb,
                                        f"pp{2 + win % 2}"):
                op()

    # loads for the v/attention part
    wv_sb = p1sb.tile([128, 8 * FH], BF16)
    for fc in range(8):
        nc.sync.dma_start(out=wv_sb[:, fc * FH:(fc + 1) * FH],
                          in_=d["wv"][fc * 128:(fc + 1) * 128, :])
    ep1 = consts.tile([128, NTOKC], F32)   # e^{+m}
    nc.sync.dma_start(out=ep1, in_=d["ep1"])
    mjb2 = consts.tile([128, NJ], F32)     # exp bias for block 2: mu*m_j
    nc.sync.dma_start(out=mjb2, in_=d["mjb2"])
    bvb = consts.tile([128, FH], F32)
    nc.sync.dma_start(out=bvb, in_=d["bvb"])
    nmr = 2 if sorted_mode else 4
    mr4 = consts.tile([nmr, N], F32)       # select rows [m, 1-m, (m, 1-m)]
    nc.sync.dma_start(out=mr4, in_=d["mr4"][0:nmr, :])
    wo_sb = consts.tile([128, 4 * 1024], BF16)
    for fc in range(4):
        nc.sync.dma_start(out=wo_sb[:, fc * 1024:(fc + 1) * 1024],
                          in_=d["wo"][fc * 128:(fc + 1) * 128, :])

    # v projection + stationaries; AV consumes s[j] at key chunk j
    vstg = p1sb.tile([128, FH], F32, tag="vstg")

    def v_ops(tokc, tagsel):
        st = {}
        ops = []
        grp = f"v{tokc}"
        for fc in range(8):
            def mm(fc=fc, tokc=tokc, st=st):
                if fc == 0:
                    st["pv"] = pools["proj"].tile([128, FH], F32,
                                                  tag=tagsel, name="pv")
                nc.tensor.matmul(
                    st["pv"][:],
                    xT_sb[:, fc * N + tokc * 128: fc * N + (tokc + 1) * 128],
                    wv_sb[:, fc * FH:(fc + 1) * FH],
                    start=(fc == 0), stop=(fc == 7))
            ops.append(("pe", grp, mm))

        def fin(tokc=tokc, st=st):
            nc.vector.tensor_add(out=vstg, in0=st["pv"][:], in1=bvb)
            base = tokc * SJ
            vv = vstg[:].rearrange("p (h c) -> p h c", h=8)
            for s_sb, scol in ((s0_sb, None), (s1_sb, ep1)):
                sv = s_sb[:, base:base + SJ].rearrange("p (h c) -> p h c", h=8)
                if scol is None:
                    nc.vector.tensor_copy(out=sv[:, :, 0:64], in_=vv)
                    nc.vector.memset(sv[:, :, 64:65], 1.0)
                else:
                    nc.vector.tensor_scalar_mul(
                        out=sv[:, :, 0:64], in0=vv,
                        scalar1=scol[:, tokc:tokc + 1])
                    colb = bass.AP(
                        tensor=scol.tensor,
                        offset=scol[:, tokc:tokc + 1].offset,
                        ap=[scol[:, tokc:tokc + 1].ap[0], [0, 8], [1, 1]])
                    nc.vector.tensor_copy(out=sv[:, :, 64:65], in_=colb)
        ops.append(("dve", grp, fin))
        return ops

    nleadv = NLEADV if sorted_mode else NTOKC
    for tokc in range(nleadv):
        for _, _, op in v_ops(tokc, f"pp{tokc % 2}"):
            op()

    # deferred work drains into attention slack (pure blocks)
    pending = []
    if sorted_mode:
        for tokc in range(NLEADV, NTOKC):
            pending.extend(v_ops(tokc, "acc1" if tokc % 2 else "acc3"))
        for fhc in (1, 2, 3):
            for win in range(4):
                pending.extend(
                    proj_qk_ops(wk_sb, fhc, win, 4 + fhc, kT_sb, "acc1"))
                pending.extend(
                    proj_qk_ops(wq_sb, fhc, win, fhc, qT_sb, "acc3"))

    pkt.release()
    if not sorted_mode:
        p1sb.release()  # nothing defers in dual mode; free xT/w space

    def drain(npe):
        done = 0
        while pending and done < npe:
            kind, _, op = pending.pop(0)
            op()
            if kind == "pe":
                done += 1
        # trailing dve ops ride along for free
        while pending and pending[0][0] == "dve":
            pending.pop(0)[2]()

    def drain_group(grp):
        """Pop (in order) until no ops of group `grp` remain."""
        while any(g == grp for _, g, _ in pending):
            pending.pop(0)[2]()

    # ================= phase 2: attention ================================
    with tc.tile_pool(name="pP", bufs=2, space="PSUM") as pP, \
         tc.tile_pool(name="pacc", bufs=1, space="PSUM") as pacc, \
         tc.tile_pool(name="sexp", bufs=2) as sexp, \
         tc.tile_pool(name="episb", bufs=2) as episb, \
         tc.tile_pool(name="rblp", bufs=1) as rblp, \
         tc.tile_pool(name="osb", bufs=2) as osb, \
         tc.tile_pool(name="epidr", bufs=2, space="DRAM") as epidr:

        pools["proj"] = pacc

        def o_ops_for_iblk(ib, tags, fin_act=False):
            ops = []
            from itertools import cycle
            tagc = cycle(tags)
            for tokc in range(ib * 4, ib * 4 + 4):
                for half in range(2):
                    st = {}
                    for fc in range(4):
                        def mm(fc=fc, tokc=tokc, half=half, st=st):
                            if fc == 0:
                                st["po"] = pacc.tile(
                                    [128, 512], F32, tag=next(tagc), name="po")
                            nc.tensor.matmul(
                                st["po"][:],
                                attnT[:, fc * N + tokc * 128: fc * N + (tokc + 1) * 128],
                                wo_sb[:, fc * 1024 + half * 512: fc * 1024 + half * 512 + 512],
                                start=(fc == 0), stop=(fc == 3))
                        ops.append(("pe", f"o{ib}", mm))

                    def fin(tokc=tokc, half=half, st=st):
                        ot = osb.tile([128, 512], F32, tag="ot", name="ot")
                        if fin_act:
                            nc.scalar.activation(
                                out=ot, in_=st["po"][:],
                                func=mybir.ActivationFunctionType.Copy)
                        else:
                            nc.vector.tensor_copy(out=ot, in_=st["po"][:])
                        nc.sync.dma_start(
                            out=d["y"][tokc * 128:(tokc + 1) * 128,
                                       half * 512:(half + 1) * 512],
                            in_=ot)
                    ops.append(("dve", f"o{ib}", fin))
            return ops

        # pure blocks first for drain slack; the dual block third
        iblk_order = [0, 2, 1, 3] if sorted_mode else [0, 1, 2, 3]
        for iblk in iblk_order:
            if sorted_mode and iblk == 0:
                active, stats = [0, 2], {0: s1_sb, 2: s1_sb}
            elif sorted_mode and iblk == 2:
                active, stats = [0, 2], {0: s0_sb, 2: s0_sb}
            elif sorted_mode and iblk == 3:
                active, stats = [1, 3], {1: s0_sb, 3: s0_sb}
            else:
                active = [0, 1, 2, 3]
                stats = {0: s1_sb, 1: s0_sb, 2: s1_sb, 3: s0_sb}
            dual = len(active) == 4
            ndrain = 6 if (sorted_mode and iblk == 0) else 2
            first_blk = iblk == iblk_order[0]
            for pair in range(NPAIR):
                if sorted_mode and pair > 0:
                    drain_group(f"f{pair}")
                accs = {
                    v: pacc.tile([65, 512], F32, tag=f"acc{v}", name=f"acc{v}")
                    for v in active
                }
                def qk(j):
                    P = pP.tile([128, 1024], F32, tag="logits")
                    for hl, tp in ((0, 0), (1, 64)):
                        nc.tensor.matmul(
                            P[:, hl * 512:(hl + 1) * 512],
                            kT_sb[tp:tp + 64, pair * N + j * 128: pair * N + (j + 1) * 128],
                            qT_sb[tp:tp + 64, pair * N + iblk * 512: pair * N + (iblk + 1) * 512],
                            start=True, stop=True, tile_position=(tp, 0))
                    return P

                P0 = qk(0)
                P1 = qk(1)
                Ptil = {0: P0, 1: P1}
                for j in range(NJ):
                    if sorted_mode and first_blk and j >= NLEADV:
                        drain_group(f"v{j}")
                    S = sexp.tile([128, 1024], BF16, tag="etil")
                    ebias = (mjb2[:, j:j + 1]
                             if (sorted_mode and iblk == 2) else 0.0)
                    nc.scalar.activation(out=S[:], in_=Ptil.pop(j),
                                         func=EXPFN, scale=1.0 / 32.0,
                                         bias=ebias)
                    if j + 2 < NJ:
                        Ptil[j + 2] = qk(j + 2)
                    for hl in range(2):
                        hcore = 2 * pair + hl
                        soff = j * SJ + hcore * SROW
                        rhs = S[:, hl * 512:(hl + 1) * 512]
                        for v in (2 * hl, 2 * hl + 1):
                            if v not in accs:
                                continue
                            nc.tensor.matmul(
                                accs[v][:], stats[v][:, soff:soff + 65], rhs,
                                start=(j == 0), stop=(j == NJ - 1))
                    if not dual:
                        drain(ndrain)

                # ---- epilogue: select + normalize -----------------------
                na = len(active)
                last_pair = iblk == iblk_order[-1] and pair == NPAIR - 1
                asb = {}
                for v in active:
                    t = episb.tile([65, 512], F32, tag=f"asb{v}", name=f"asb{v}")
                    if last_pair:
                        nc.scalar.activation(
                            out=t, in_=accs[v][:],
                            func=mybir.ActivationFunctionType.Copy)
                    else:
                        nc.vector.tensor_copy(out=t, in_=accs[v][:])
                    asb[v] = t
                # reciprocal rows: head-group A (v 0/1) and B (v 2/3) in
                # separate partition-base-0 tiles so the 2-row select works
                rinA = episb.tile([2, 512], F32, tag="rinA")
                rinB = episb.tile([2, 512], F32, tag="rinB")
                rtile = {v: ((rinA, rinB)[v // 2], v % 2 if dual else 0)
                         for v in active}
                for v in active:
                    t, r = rtile[v]
                    nc.sync.dma_start(out=t[r:r + 1, :], in_=asb[v][64:65, :])
                nra = 2 if dual else 1
                nc.vector.reciprocal_approx_fast(out=rinA[0:nra, :],
                                                 in_=rinA[0:nra, :])
                nc.vector.reciprocal_approx_fast(out=rinB[0:nra, :],
                                                 in_=rinB[0:nra, :])
                if dual:
                    ib = iblk * 512
                    nc.vector.tensor_mul(out=rinA[:], in0=rinA[:],
                                         in1=mr4[0:2, ib:ib + 512])
                    nc.vector.tensor_mul(out=rinB[:], in0=rinB[:],
                                         in1=mr4[0:2, ib:ib + 512])
                stg2 = epidr.tile([4, 512], F32, tag="stg2")
                for k, v in enumerate(active):
                    t, r = rtile[v]
                    nc.sync.dma_start(out=stg2[k:k + 1, :], in_=t[r:r + 1, :])
                rball = rblp.tile([64, 4 * 512], F32, tag="rball")
                nc.sync.dma_start(
                    out=rball[:, 0:na * 512],
                    in_=bass.AP(tensor=stg2.tensor, offset=stg2.offset,
                                ap=[[0, 64], [512, na], [1, 512]]))
                rb = {v: rball[:, k * 512:(k + 1) * 512]
                      for k, v in enumerate(active)}
                for hl in range(2):
                    dstc = pair * N + iblk * 512
                    v1, v0 = 2 * hl, 2 * hl + 1
                    if dual:
                        t1 = episb.tile([64, 512], F32, tag="ept1")
                        t2 = episb.tile([64, 512], F32, tag="ept2")
                        nc.vector.tensor_mul(out=t1, in0=asb[v1][0:64, :], in1=rb[v1])
                        nc.vector.tensor_mul(out=t2, in0=asb[v0][0:64, :], in1=rb[v0])
                        if hl == 0:
                            nc.vector.tensor_add(
                                out=attnT[0:64, dstc:dstc + 512], in0=t1, in1=t2)
                        else:
                            t3 = episb.tile([64, 512], BF16, tag="ept3")
                            nc.vector.tensor_add(out=t3, in0=t1, in1=t2)
                            nc.sync.dma_start(
                                out=attnT[64:128, dstc:dstc + 512], in_=t3)
                    else:
                        vv = v1 if v1 in asb else v0
                        if hl == 0:
                            nc.vector.tensor_mul(
                                out=attnT[0:64, dstc:dstc + 512],
                                in0=asb[vv][0:64, :], in1=rb[vv])
                        else:
                            t3 = episb.tile([64, 512], BF16, tag="ept3")
                            nc.vector.tensor_mul(out=t3, in0=asb[vv][0:64, :],
                                                 in1=rb[vv])
                            nc.sync.dma_start(
                                out=attnT[64:128, dstc:dstc + 512], in_=t3)

            if sorted_mode:
                # tags match the free accs of the block where the ops DRAIN:
                # o(0) drains in blk2 {0,2 active}; o(2), o(1) in blk3
                # {1,3 active}; o(3) at the tail.
                tags = ("acc1", "acc3") if iblk == 0 else ("acc0", "acc2")
                pending.extend(
                    o_ops_for_iblk(iblk, tags, fin_act=(iblk == iblk_order[-1])))
            else:
                pending.extend(o_ops_for_iblk(iblk, ("acc0", "acc2"),
                                              fin_act=True))

        # ===== tail: drain remaining ops, keep PE warm =====================
        if sorted_mode:
            warm2 = pacc.tile([128, 512], F32, tag="acc1", name="warm2")
            for _ in range(14):
                nc.tensor.matmul(warm2[:], wo_sb[:, 0:128], wo_sb[:, 0:512],
                                 start=True, stop=True)
        while pending:
            pending.pop(0)[2]()

    if sorted_mode:
        p1sb.release()
    persist.release()
    consts.release()


_CACHE = {}


def build_program(variant="sorted"):
    if variant in _CACHE:
        return _CACHE[variant]
    nc = bacc.Bacc("TRN2", target_bir_lowering=False, debug=False)
    d = {}
    d["xT"] = nc.dram_tensor("xT", (F, N), BF16, kind="ExternalInput").ap()
    d["wq"] = nc.dram_tensor("wq", (F, FH), BF16, kind="ExternalInput").ap()
    d["wk"] = nc.dram_tensor("wk", (F, FH), BF16, kind="ExternalInput").ap()
    d["wv"] = nc.dram_tensor("wv", (F, FH), BF16, kind="ExternalInput").ap()
    d["wo"] = nc.dram_tensor("wo", (FH, F), BF16, kind="ExternalInput").ap()
    d["bqk"] = nc.dram_tensor("bqk", (128, 8), F32, kind="ExternalInput").ap()
    d["bvb"] = nc.dram_tensor("bvb", (128, FH), F32, kind="ExternalInput").ap()
    d["ep1"] = nc.dram_tensor("ep1", (128, NTOKC), F32, kind="ExternalInput").ap()
    d["mjb2"] = nc.dram_tensor("mjb2", (128, NJ), F32, kind="ExternalInput").ap()
    d["mr4"] = nc.dram_tensor("mr4", (4, N), F32, kind="ExternalInput").ap()
    d["y"] = nc.dram_tensor("y", (N, F), F32, kind="ExternalOutput").ap()
    with tile.TileContext(nc) as tc:
        _emit(nc, tc, d, sorted_mode=(variant == "sorted"))
    nc.compile()
    _CACHE[variant] = nc
    return nc


def _perm_blocks(m):
    """Permutation putting tokens into blocks: 0 pure-1, 1 mixed, 2 pure
    (mu = c1 > 1024), 3 pure-0. Returns perm, mu."""
    ones = np.flatnonzero(m > 0.5)
    zeros = np.flatnonzero(m <= 0.5)
    c1 = len(ones)
    if c1 > 1024:
        mu = 1.0
        perm = np.concatenate([
            ones[0:512], ones[1024:], zeros[0:1536 - c1],
            ones[512:1024], zeros[1536 - c1:]])
    else:
        mu = 0.0
        perm = np.concatenate([
            ones[0:512], ones[512:], zeros[0:1024 - c1],
            zeros[1024 - c1:1536 - c1], zeros[1536 - c1:]])
    return perm, mu


def make_in_maps(x, inputs_mask, Wq, bq, Wk, bk, Wv, bv, Wo, bo,
                 sorted_mode=True):
    in_maps = []
    m_all = inputs_mask.astype(np.float32)
    perms, mus = [], []
    for b in range(B):
        if sorted_mode:
            p, mu = _perm_blocks(m_all[b])
        else:
            p, mu = np.arange(N), 0.0
        perms.append(p)
        mus.append(mu)
    for c in range(NC_):
        b, hh = c // 2, c % 2
        cs = slice(hh * FH, (hh + 1) * FH)
        m = m_all[b][perms[b]]
        xb = x[b][perms[b]]
        im = {
            "xT": np.ascontiguousarray(xb.T).astype(NPBF16),
            "wq": Wq[:, cs].astype(NPBF16),
            "wk": Wk[:, cs].astype(NPBF16),
            "wv": Wv[:, cs].astype(NPBF16),
            "wo": np.ascontiguousarray(Wo[cs, :]).astype(NPBF16),
            "bqk": np.stack(
                [bq[cs].reshape(4, 128), bk[cs].reshape(4, 128)], axis=0
            ).reshape(8, 128).T.astype(np.float32).copy(),
            "bvb": np.broadcast_to(bv[cs], (128, FH)).astype(np.float32).copy(),
            "ep1": np.exp(m).reshape(NTOKC, 128).T.astype(np.float32).copy(),
            "mjb2": (mus[b] * m).reshape(NJ, 128).T.astype(np.float32).copy(),
            "mr4": np.stack([m, 1.0 - m, m, 1.0 - m]).astype(np.float32).copy(),
        }
        in_maps.append(im)
    return in_maps, perms


def kernel(x, inputs_mask, Wq, bq, Wk, bk, Wv, bv, Wo, bo):
    x = np.asarray(x, dtype=np.float32)
    inputs_mask = np.asarray(inputs_mask)
    Wq, bq = np.asarray(Wq, np.float32), np.asarray(bq, np.float32)
    Wk, bk = np.asarray(Wk, np.float32), np.asarray(bk, np.float32)
    Wv, bv = np.asarray(Wv, np.float32), np.asarray(bv, np.float32)
    Wo, bo = np.asarray(Wo, np.float32), np.asarray(bo, np.float32)

    c1 = inputs_mask.astype(np.int64).sum(axis=1)
    sorted_mode = bool(np.all((c1 >= 512) & (c1 <= 3 * 512)))
    nc = build_program("sorted" if sorted_mode else "dual")
    in_maps, perms = make_in_maps(
        x, inputs_mask, Wq, bq, Wk, bk, Wv, bv, Wo, bo, sorted_mode=sorted_mode)
    res = bass_utils.run_bass_kernel_spmd(nc, in_maps, core_ids=list(range(NC_)))
    out = np.empty((B, N, F), dtype=np.float32)
    for b in range(B):
        out[b][perms[b]] = (
            res.results[2 * b]["y"] + res.results[2 * b + 1]["y"] + bo
        )
    return out


# revision 20
# speedup vs baseline: 1.0155x; 1.0107x over previous
"""Multi-head self-attention (b=4, n=2048, f=1024, h=16) on 8 trn2 NeuronCores.

Sharding: core c -> batch c//2, head-half c%2 (8 heads of 64 dims each).
Each core computes its 8 heads' attention and a partial output projection
(attn_slice @ Wo_rows); host sums the two partials per batch and adds bo.

v3 over the original kernel (all matmul operands bf16, PSUM fp32):
  - exp has NO bias: the additive mask term exp(m_j) is folded into the AV
    stationaries instead (s1 = e^{+m}[v|1], s0 = [v|1]). This frees the
    choice of AV stationary per query block.
  - the host permutation places tokens so query blocks 0/2/3 are pure
    (block 2's mask value mu arrives as data via a third stationary
    s2 = e^{mu m}[v|1]); only block 1 needs the dual-variant select.
    AV work: 10 matmuls per (pair, key-chunk) vs 12 before.
  - the exp stream on ScalarE (256 x [128,1024] chunks at ~1.15us each,
    ~295us total) is the critical path; PE work (~310us) runs just under
    it. Phase 1 is interleaved into the attention stream: only kT/qT
    chunk 0 and half of v run up front (~30us); the rest of v, kT/qT
    chunks 1-3, and the O-projection drain as micro-ops into the PE slack
    of the pure query blocks, so ScalarE starts exp'ing as early as
    possible and never waits.
"""

import sys

sys.path.insert(0, "/opt/trn_rl_repo")

import numpy as np
import ml_dtypes

import concourse.bass as bass
import concourse.bacc as bacc
import concourse.mybir as mybir
import concourse.tile as tile
from concourse import bass_utils

BF16 = mybir.dt.bfloat16
F32 = mybir.dt.float32
NPBF16 = ml_dtypes.bfloat16

B, N, F, H, HD = 4, 2048, 1024, 16, 64
FH = 512          # features per core (8 heads)
NC_ = 8           # cores
NTOKC = N // 128  # 16 token chunks
NIBLK = N // 512  # 4 query blocks
NJ = N // 128     # 16 key chunks
NPAIR = 4         # head pairs per core
EXPFN = mybir.ActivationFunctionType.Exp
SROW = 66         # stationary cols per head
SJ = 8 * SROW     # stationary cols per key chunk (528)
NLEADV = 8        # v token-chunks computed up front; the rest drain


def _emit(nc, tc, d, sorted_mode):
    consts = tc.alloc_tile_pool(name="consts", bufs=1)
    persist = tc.alloc_tile_pool(name="persist", bufs=1)

    # ---- persistent activations ----------------------------------------
    qT_sb = persist.tile([128, 4 * N], BF16)   # [feat, tok], fhc at cols fhc*N
    kT_sb = persist.tile([128, 4 * N], BF16)
    s0_sb = persist.tile([128, NJ * SJ + 64], BF16)  # [j][h][66]: [vb|1]
    s1_sb = persist.tile([128, NJ * SJ + 64], BF16)  # e^{+m} * [vb|1]
    attnT = persist.tile([128, 4 * N], BF16)   # normalized attn, [feat, tok]

    # ================= phase 1 (lead-in part) ============================
    p1sb = tc.alloc_tile_pool(name="p1sb", bufs=1)
    pkt = tc.alloc_tile_pool(name="pkt", bufs=1, space="PSUM")

    bqk = consts.tile([128, 8], F32)       # bq chunks (0-3), bk (4-7)
    nc.sync.dma_start(out=bqk, in_=d["bqk"])
    nc.vector.memset(s0_sb[:], 0.0)
    nc.vector.memset(s1_sb[:], 0.0)
    # exp table warm-up (~2.7us) while DMAs run
    warm = consts.tile([128, 8], BF16)
    nc.scalar.activation(out=warm, in_=bqk, func=EXPFN, scale=0.0)

    xT_sb = p1sb.tile([128, 8 * N], BF16)
    wk_sb = p1sb.tile([128, 8 * FH], BF16)
    wq_sb = p1sb.tile([128, 8 * FH], BF16)
    for fc in range(8):
        nc.sync.dma_start(out=wk_sb[:, fc * FH:(fc + 1) * FH],
                          in_=d["wk"][fc * 128:(fc + 1) * 128, :])
        nc.sync.dma_start(out=xT_sb[:, fc * N:(fc + 1) * N],
                          in_=d["xT"][fc * 128:(fc + 1) * 128, :])
    for fc in range(8):
        nc.sync.dma_start(out=wq_sb[:, fc * FH:(fc + 1) * FH],
                          in_=d["wq"][fc * 128:(fc + 1) * 128, :])

    pools = {"proj": pkt}  # phase 2 rebinds this to its own PSUM pool

    def proj_qk_ops(w_sb, fhc, win, bias_col, out_sb, tagsel):
        """Micro-ops for one [128,512] window of a q/k projection.

        Returns list of ("pe"|"dve", fn) micro-ops: 8 accumulation matmuls
        plus the bias-add into the bf16 destination.
        """
        st = {}
        ops = []
        grp = f"f{fhc}"
        for fc in range(8):
            def mm(fc=fc, st=st):
                if fc == 0:
                    st["pk"] = pools["proj"].tile([128, 512], F32,
                                                  tag=tagsel, name="pk")
                nc.tensor.matmul(
                    st["pk"][:],
                    w_sb[:, fc * FH + fhc * 128: fc * FH + (fhc + 1) * 128],
                    xT_sb[:, fc * N + win * 512: fc * N + (win + 1) * 512],
                    start=(fc == 0), stop=(fc == 7))
            ops.append(("pe", grp, mm))

        def fin(st=st):
            nc.vector.tensor_scalar_add(
                out=out_sb[:, fhc * N + win * 512: fhc * N + win * 512 + 512],
                in0=st["pk"][:], scalar1=bqk[:, bias_col:bias_col + 1])
        ops.append(("dve", grp, fin))
        return ops

    # kT/qT chunk 0 up front (pair 0 of the first query block)
    lead_fhc = (0,) if sorted_mode else (0, 1, 2, 3)
    for fhc in lead_fhc:
        for win in range(4):
            for _, _, op in proj_qk_ops(wk_sb, fhc, win, 4 + fhc, kT_sb,
                                        f"pp{win % 2}"):
                op()
            for _, _, op in proj_qk_ops(wq_sb, fhc, win, fhc, qT_sb,
                                        f"pp{2 + win % 2}"):
                op()

    # loads for the v/attention part
    wv_sb = p1sb.tile([128, 8 * FH], BF16)
    for fc in range(8):
        nc.sync.dma_start(out=wv_sb[:, fc * FH:(fc + 1) * FH],
                          in_=d["wv"][fc * 128:(fc + 1) * 128, :])
    ep1 = consts.tile([128, NTOKC], F32)   # e^{+m}
    nc.sync.dma_start(out=ep1, in_=d["ep1"])
    mjb2 = consts.tile([128, NJ], F32)     # exp bias for block 2: mu*m_j
    nc.sync.dma_start(out=mjb2, in_=d["mjb2"])
    bvb = consts.tile([128, FH], F32)
    nc.sync.dma_start(out=bvb, in_=d["bvb"])
    nmr = 2 if sorted_mode else 4
    mr4 = consts.tile([nmr, N], F32)       # select rows [m, 1-m, (m, 1-m)]
    nc.sync.dma_start(out=mr4, in_=d["mr4"][0:nmr, :])
    wo_sb = consts.tile([128, 4 * 1024], BF16)
    for fc in range(4):
        nc.sync.dma_start(out=wo_sb[:, fc * 1024:(fc + 1) * 1024],
                          in_=d["wo"][fc * 128:(fc + 1) * 128, :])

    # v projection + stationaries; AV consumes s[j] at key chunk j
    vstg = p1sb.tile([128, FH], F32, tag="vstg")

    def v_ops(tokc, tagsel):
        st = {}
        ops = []
        grp = f"v{tokc}"
        for fc in range(8):
            def mm(fc=fc, tokc=tokc, st=st):
                if fc == 0:
                    st["pv"] = pools["proj"].tile([128, FH], F32,
                                                  tag=tagsel, name="pv")
                nc.tensor.matmul(
                    st["pv"][:],
                    xT_sb[:, fc * N + tokc * 128: fc * N + (tokc + 1) * 128],
                    wv_sb[:, fc * FH:(fc + 1) * FH],
                    start=(fc == 0), stop=(fc == 7))
            ops.append(("pe", grp, mm))

        def fin(tokc=tokc, st=st):
            nc.vector.tensor_add(out=vstg, in0=st["pv"][:], in1=bvb)
            base = tokc * SJ
            vv = vstg[:].rearrange("p (h c) -> p h c", h=8)
            for s_sb, scol in ((s0_sb, None), (s1_sb, ep1)):
                sv = s_sb[:, base:base + SJ].rearrange("p (h c) -> p h c", h=8)
                if scol is None:
                    nc.vector.tensor_copy(out=sv[:, :, 0:64], in_=vv)
                    nc.vector.memset(sv[:, :, 64:65], 1.0)
                else:
                    nc.vector.tensor_scalar_mul(
                        out=sv[:, :, 0:64], in0=vv,
                        scalar1=scol[:, tokc:tokc + 1])
                    colb = bass.AP(
                        tensor=scol.tensor,
                        offset=scol[:, tokc:tokc + 1].offset,
                        ap=[scol[:, tokc:tokc + 1].ap[0], [0, 8], [1, 1]])
                    nc.vector.tensor_copy(out=sv[:, :, 64:65], in_=colb)
        ops.append(("dve", grp, fin))
        return ops

    nleadv = NLEADV if sorted_mode else NTOKC
    for tokc in range(nleadv):
        for _, _, op in v_ops(tokc, f"pp{tokc % 2}"):
            op()

    # deferred work drains into attention slack (pure blocks)
    pending = []
    if sorted_mode:
        for tokc in range(NLEADV, NTOKC):
            pending.extend(v_ops(tokc, "acc1" if tokc % 2 else "acc3"))
        for fhc in (1, 2, 3):
            for win in range(4):
                pending.extend(
                    proj_qk_ops(wk_sb, fhc, win, 4 + fhc, kT_sb, "acc1"))
                pending.extend(
                    proj_qk_ops(wq_sb, fhc, win, fhc, qT_sb, "acc3"))

    pkt.release()
    if not sorted_mode:
        p1sb.release()  # nothing defers in dual mode; free xT/w space

    def drain(npe):
        done = 0
        while pending and done < npe:
            kind, _, op = pending.pop(0)
            op()
            if kind == "pe":
                done += 1
        # trailing dve ops ride along for free
        while pending and pending[0][0] == "dve":
            pending.pop(0)[2]()

    def drain_group(grp):
        """Pop (in order) until no ops of group `grp` remain."""
        while any(g == grp for _, g, _ in pending):
            pending.pop(0)[2]()

    # ================= phase 2: attention ================================
    with tc.tile_pool(name="pP", bufs=2, space="PSUM") as pP, \
         tc.tile_pool(name="pacc", bufs=1, space="PSUM") as pacc, \
         tc.tile_pool(name="sexp", bufs=2) as sexp, \
         tc.tile_pool(name="episb", bufs=2) as episb, \
         tc.tile_pool(name="rblp", bufs=1) as rblp, \
         tc.tile_pool(name="osb", bufs=2) as osb, \
         tc.tile_pool(name="epidr", bufs=2, space="DRAM") as epidr:

        pools["proj"] = pacc

        def o_ops_for_iblk(ib, tags, fin_act=False):
            ops = []
            from itertools import cycle
            tagc = cycle(tags)
            for tokc in range(ib * 4, ib * 4 + 4):
                for half in range(2):
                    st = {}
                    for fc in range(4):
                        def mm(fc=fc, tokc=tokc, half=half, st=st):
                            if fc == 0:
                                st["po"] = pacc.tile(
                                    [128, 512], F32, tag=next(tagc), name="po")
                            nc.tensor.matmul(
                                st["po"][:],
                                attnT[:, fc * N + tokc * 128: fc * N + (tokc + 1) * 128],
                                wo_sb[:, fc * 1024 + half * 512: fc * 1024 + half * 512 + 512],
                                start=(fc == 0), stop=(fc == 3))
                        ops.append(("pe", f"o{ib}", mm))

                    def fin(tokc=tokc, half=half, st=st):
                        ot = osb.tile([128, 512], F32, tag="ot", name="ot")
                        if fin_act:
                            nc.scalar.activation(
                                out=ot, in_=st["po"][:],
                                func=mybir.ActivationFunctionType.Copy)
                        else:
                            nc.vector.tensor_copy(out=ot, in_=st["po"][:])
                        nc.sync.dma_start(
                            out=d["y"][tokc * 128:(tokc + 1) * 128,
                                       half * 512:(half + 1) * 512],
                            in_=ot)
                    ops.append(("dve", f"o{ib}", fin))
            return ops

        # pure blocks first for drain slack; the dual block third
        iblk_order = [0, 2, 1, 3] if sorted_mode else [0, 1, 2, 3]
        for iblk in iblk_order:
            if sorted_mode and iblk == 0:
                active, stats = [0, 2], {0: s1_sb, 2: s1_sb}
            elif sorted_mode and iblk == 2:
                active, stats = [0, 2], {0: s0_sb, 2: s0_sb}
            elif sorted_mode and iblk == 3:
                active, stats = [1, 3], {1: s0_sb, 3: s0_sb}
            else:
                active = [0, 1, 2, 3]
                stats = {0: s1_sb, 1: s0_sb, 2: s1_sb, 3: s0_sb}
            dual = len(active) == 4
            ndrain = 6 if (sorted_mode and iblk == 0) else 2
            first_blk = iblk == iblk_order[0]
            for pair in range(NPAIR):
                if sorted_mode and pair > 0:
                    drain_group(f"f{pair}")
                accs = {
                    v: pacc.tile([128, 512], F32, tag=f"acc{v}", name=f"acc{v}")
                    for v in active
                }
                def qk(j):
                    P = pP.tile([128, 1024], F32, tag="logits")
                    for hl, tp in ((0, 0), (1, 64)):
                        nc.tensor.matmul(
                            P[:, hl * 512:(hl + 1) * 512],
                            kT_sb[tp:tp + 64, pair * N + j * 128: pair * N + (j + 1) * 128],
                            qT_sb[tp:tp + 64, pair * N + iblk * 512: pair * N + (iblk + 1) * 512],
                            start=True, stop=True, tile_position=(tp, 0))
                    return P

                P0 = qk(0)
                P1 = qk(1)
                Ptil = {0: P0, 1: P1}
                for j in range(NJ):
                    if sorted_mode and first_blk and j >= NLEADV:
                        drain_group(f"v{j}")
                    S = sexp.tile([128, 1024], BF16, tag="etil")
                    ebias = (mjb2[:, j:j + 1]
                             if (sorted_mode and iblk == 2) else 0.0)
                    nc.scalar.activation(out=S[:], in_=Ptil.pop(j),
                                         func=EXPFN, scale=1.0 / 32.0,
                                         bias=ebias)
                    if j + 2 < NJ:
                        Ptil[j + 2] = qk(j + 2)
                    for hl in range(2):
                        hcore = 2 * pair + hl
                        soff = j * SJ + hcore * SROW
                        rhs = S[:, hl * 512:(hl + 1) * 512]
                        for v in (2 * hl, 2 * hl + 1):
                            if v not in accs:
                                continue
                            # 128-wide stationary read (cols 65+ produce
                            # ignored partitions) keeps FWL enabled
                            nc.tensor.matmul(
                                accs[v][:], stats[v][:, soff:soff + 128], rhs,
                                start=(j == 0), stop=(j == NJ - 1))
                    if not dual:
                        drain(ndrain)

                # ---- epilogue: select + normalize -----------------------
                na = len(active)
                last_pair = iblk == iblk_order[-1] and pair == NPAIR - 1
                asb = {}
                for v in active:
                    t = episb.tile([65, 512], F32, tag=f"asb{v}", name=f"asb{v}")
                    if last_pair:
                        nc.scalar.activation(
                            out=t, in_=accs[v][0:65, :],
                            func=mybir.ActivationFunctionType.Copy)
                    else:
                        nc.vector.tensor_copy(out=t, in_=accs[v][0:65, :])
                    asb[v] = t
                # reciprocal rows: head-group A (v 0/1) and B (v 2/3) in
                # separate partition-base-0 tiles so the 2-row select works
                rinA = episb.tile([2, 512], F32, tag="rinA")
                rinB = episb.tile([2, 512], F32, tag="rinB")
                rtile = {v: ((rinA, rinB)[v // 2], v % 2 if dual else 0)
                         for v in active}
                for v in active:
                    t, r = rtile[v]
                    nc.sync.dma_start(out=t[r:r + 1, :], in_=asb[v][64:65, :])
                nra = 2 if dual else 1
                nc.vector.reciprocal_approx_fast(out=rinA[0:nra, :],
                                                 in_=rinA[0:nra, :])
                nc.vector.reciprocal_approx_fast(out=rinB[0:nra, :],
                                                 in_=rinB[0:nra, :])
                if dual:
                    ib = iblk * 512
                    nc.vector.tensor_mul(out=rinA[:], in0=rinA[:],
                                         in1=mr4[0:2, ib:ib + 512])
                    nc.vector.tensor_mul(out=rinB[:], in0=rinB[:],
                                         in1=mr4[0:2, ib:ib + 512])
                stg2 = epidr.tile([4, 512], F32, tag="stg2")
                for k, v in enumerate(active):
                    t, r = rtile[v]
                    nc.sync.dma_start(out=stg2[k:k + 1, :], in_=t[r:r + 1, :])
                rball = rblp.tile([64, 4 * 512], F32, tag="rball")
                nc.sync.dma_start(
                    out=rball[:, 0:na * 512],
                    in_=bass.AP(tensor=stg2.tensor, offset=stg2.offset,
                                ap=[[0, 64], [512, na], [1, 512]]))
                rb = {v: rball[:, k * 512:(k + 1) * 512]
                      for k, v in enumerate(active)}
                for hl in range(2):
                    dstc = pair * N + iblk * 512
                    v1, v0 = 2 * hl, 2 * hl + 1
                    if dual:
                        t1 = episb.tile([64, 512], F32, tag="ept1")
                        t2 = episb.tile([64, 512], F32, tag="ept2")
                        nc.vector.tensor_mul(out=t1, in0=asb[v1][0:64, :], in1=rb[v1])
                        nc.vector.tensor_mul(out=t2, in0=asb[v0][0:64, :], in1=rb[v0])
                        if hl == 0:
                            nc.vector.tensor_add(
                                out=attnT[0:64, dstc:dstc + 512], in0=t1, in1=t2)
                        else:
                            t3 = episb.tile([64, 512], BF16, tag="ept3")
                            nc.vector.tensor_add(out=t3, in0=t1, in1=t2)
                            nc.sync.dma_start(
                                out=attnT[64:128, dstc:dstc + 512], in_=t3)
                    else:
                        vv = v1 if v1 in asb else v0
                        if hl == 0:
                            nc.vector.tensor_mul(
                                out=attnT[0:64, dstc:dstc + 512],
                                in0=asb[vv][0:64, :], in1=rb[vv])
                        else:
                            t3 = episb.tile([64, 512], BF16, tag="ept3")
                            nc.vector.tensor_mul(out=t3, in0=asb[vv][0:64, :],
                                                 in1=rb[vv])
                            nc.sync.dma_start(
                                out=attnT[64:128, dstc:dstc + 512], in_=t3)

            if sorted_mode:
                # tags match the free accs of the block where the ops DRAIN:
                # o(0) drains in blk2 {0,2 active}; o(2), o(1) in blk3
                # {1,3 active}; o(3) at the tail.
                tags = ("acc1", "acc3") if iblk == 0 else ("acc0", "acc2")
                pending.extend(
                    o_ops_for_iblk(iblk, tags, fin_act=(iblk == iblk_order[-1])))
            else:
                pending.extend(o_ops_for_iblk(iblk, ("acc0", "acc2"),
                                              fin_act=True))

        # ===== tail: drain remaining ops, keep PE warm =====================
        if sorted_mode:
            warm2 = pacc.tile([128, 512], F32, tag="acc1", name="warm2")
            for _ in range(14):
                nc.tensor.matmul(warm2[:], wo_sb[:, 0:128], wo_sb[:, 0:512],
                                 start=True, stop=True)
        while pending:
            pending.pop(0)[2]()

    if sorted_mode:
        p1sb.release()
    persist.release()
    consts.release()


_CACHE = {}


def build_program(variant="sorted"):
    if variant in _CACHE:
        return _CACHE[variant]
    nc = bacc.Bacc("TRN2", target_bir_lowering=False, debug=False)
    d = {}
    d["xT"] = nc.dram_tensor("xT", (F, N), BF16, kind="ExternalInput").ap()
    d["wq"] = nc.dram_tensor("wq", (F, FH), BF16, kind="ExternalInput").ap()
    d["wk"] = nc.dram_tensor("wk", (F, FH), BF16, kind="ExternalInput").ap()
    d["wv"] = nc.dram_tensor("wv", (F, FH), BF16, kind="ExternalInput").ap()
    d["wo"] = nc.dram_tensor("wo", (FH, F), BF16, kind="ExternalInput").ap()
    d["bqk"] = nc.dram_tensor("bqk", (128, 8), F32, kind="ExternalInput").ap()
    d["bvb"] = nc.dram_tensor("bvb", (128, FH), F32, kind="ExternalInput").ap()
    d["ep1"] = nc.dram_tensor("ep1", (128, NTOKC), F32, kind="ExternalInput").ap()
    d["mjb2"] = nc.dram_tensor("mjb2", (128, NJ), F32, kind="ExternalInput").ap()
    d["mr4"] = nc.dram_tensor("mr4", (4, N), F32, kind="ExternalInput").ap()
    d["y"] = nc.dram_tensor("y", (N, F), F32, kind="ExternalOutput").ap()
    with tile.TileContext(nc) as tc:
        _emit(nc, tc, d, sorted_mode=(variant == "sorted"))
    nc.compile()
    _CACHE[variant] = nc
    return nc


def _perm_blocks(m):
    """Permutation putting tokens into blocks: 0 pure-1, 1 mixed, 2 pure
    (mu = c1 > 1024), 3 pure-0. Returns perm, mu."""
    ones = np.flatnonzero(m > 0.5)
    zeros = np.flatnonzero(m <= 0.5)
    c1 = len(ones)
    if c1 > 1024:
        mu = 1.0
        perm = np.concatenate([
            ones[0:512], ones[1024:], zeros[0:1536 - c1],
            ones[512:1024], zeros[1536 - c1:]])
    else:
        mu = 0.0
        perm = np.concatenate([
            ones[0:512], ones[512:], zeros[0:1024 - c1],
            zeros[1024 - c1:1536 - c1], zeros[1536 - c1:]])
    return perm, mu


def make_in_maps(x, inputs_mask, Wq, bq, Wk, bk, Wv, bv, Wo, bo,
                 sorted_mode=True):
    in_maps = []
    m_all = inputs_mask.astype(np.float32)
    perms, mus = [], []
    for b in range(B):
        if sorted_mode:
            p, mu = _perm_blocks(m_all[b])
        else:
            p, mu = np.arange(N), 0.0
        perms.append(p)
        mus.append(mu)
    for c in range(NC_):
        b, hh = c // 2, c % 2
        cs = slice(hh * FH, (hh + 1) * FH)
        m = m_all[b][perms[b]]
        xb = x[b][perms[b]]
        im = {
            "xT": np.ascontiguousarray(xb.T).astype(NPBF16),
            "wq": Wq[:, cs].astype(NPBF16),
            "wk": Wk[:, cs].astype(NPBF16),
            "wv": Wv[:, cs].astype(NPBF16),
            "wo": np.ascontiguousarray(Wo[cs, :]).astype(NPBF16),
            "bqk": np.stack(
                [bq[cs].reshape(4, 128), bk[cs].reshape(4, 128)], axis=0
            ).reshape(8, 128).T.astype(np.float32).copy(),
            "bvb": np.broadcast_to(bv[cs], (128, FH)).astype(np.float32).copy(),
            "ep1": np.exp(m).reshape(NTOKC, 128).T.astype(np.float32).copy(),
            "mjb2": (mus[b] * m).reshape(NJ, 128).T.astype(np.float32).copy(),
            "mr4": np.stack([m, 1.0 - m, m, 1.0 - m]).astype(np.float32).copy(),
        }
        in_maps.append(im)
    return in_maps, perms


def kernel(x, inputs_mask, Wq, bq, Wk, bk, Wv, bv, Wo, bo):
    x = np.asarray(x, dtype=np.float32)
    inputs_mask = np.asarray(inputs_mask)
    Wq, bq = np.asarray(Wq, np.float32), np.asarray(bq, np.float32)
    Wk, bk = np.asarray(Wk, np.float32), np.asarray(bk, np.float32)
    Wv, bv = np.asarray(Wv, np.float32), np.asarray(bv, np.float32)
    Wo, bo = np.asarray(Wo, np.float32), np.asarray(bo, np.float32)

    c1 = inputs_mask.astype(np.int64).sum(axis=1)
    sorted_mode = bool(np.all((c1 >= 512) & (c1 <= 3 * 512)))
    nc = build_program("sorted" if sorted_mode else "dual")
    in_maps, perms = make_in_maps(
        x, inputs_mask, Wq, bq, Wk, bk, Wv, bv, Wo, bo, sorted_mode=sorted_mode)
    res = bass_utils.run_bass_kernel_spmd(nc, in_maps, core_ids=list(range(NC_)))
    out = np.empty((B, N, F), dtype=np.float32)
    for b in range(B):
        out[b][perms[b]] = (
            res.results[2 * b]["y"] + res.results[2 * b + 1]["y"] + bo
        )
    return out
